# revision 42
# baseline (speedup 1.0000x reference)
"""Trainium2 Bass kernel for a Mixtral decoder layer (attention + top-2 MoE).

3-launch pipeline over 8 NeuronCores, all heavy matmuls in fp8e4 DoubleRow
(2 fp8 weights/PE-cell, 256-wide contraction, 0.5 cyc/row):

  L1: attention(batch 0), 8-way head-sharded (2 heads / core). Each core
      computes q/k/v projections for its 128-wide head slice, exp-softmax
      scores (ACT-bound), AV with a ones-column denominator (M=65 DoubleRow),
      and a partial O-projection -> h1p [S, H] bf16. Host sums partials.
  host: residual + rmsnorm + exact fp32/64 top-2 routing for batch 0 (free).
  L2: attention(batch 1) + expert-parallel MoE FFN(batch 0) interleaved on
      the same cores: MoE matmul slices fill the PE idle time under the
      ACT exp shadow.
  host: routing for batch 1.
  L3: MoE FFN(batch 1).

Scales: weights are pre-scaled into fp8-friendly ranges on the host
(lam=64 for wq/wk/wv/wo/w1/w2, lam=8 for w3) and descaled on-device by
folding into the exp scale / silu scale / output copies.
"""
import os
import sys
from collections import defaultdict

import numpy as np
import ml_dtypes

for _p in ("/root/.axon_site", "/root/.axon_site/_ro/trn_rl_repo", "/opt/trn_rl_repo"):
    if os.path.isdir(_p) and _p not in sys.path:
        sys.path.append(_p)

import concourse.tile as tile
from concourse import bacc, mybir
from concourse.bass_utils import run_bass_kernel_spmd

BF16 = ml_dtypes.bfloat16
F8 = ml_dtypes.float8_e4m3fn
AF = mybir.ActivationFunctionType
ALU = mybir.AluOpType
DT = mybir.dt
PM = mybir.MatmulPerfMode

H = 1024
S = 2048
B = 2
NH = 16
D = 64
E = 8
I = 2048
T = B * S
EPS = 1e-5
NCORES = 8

NCI = H // 128        # 8 contraction chunks of the hidden dim
NTK = S // 128        # 16 k-tiles
NIC = I // 128        # 16 chunks of the FFN dim

LAM = 64.0            # wq/wk/wv/wo/w1/w2 fp8 scale
LAM3 = 8.0            # w3 fp8 scale
EXP_SCALE = 0.125 / (LAM * LAM)
OPROJ_SCALE = 1.0 / (LAM * LAM)   # ao carries LAM, wo8 carries LAM
Y_SCALE = 1.0 / (LAM3 * LAM * LAM)  # hh carries LAM3*? see emit_moe

_CACHE = {}
LAST_RESULTS = []
TRACE = os.environ.get("KERNEL_TRACE", "0") == "1"


def _capacity_chunks(cap):
    out, o = [], 0
    while o < cap:
        ln = min(512, cap - o)
        out.append((o, ln))
        o += ln
    return out


class _MoeEmitter:
    """MoE FFN for one expert-core: load setup + compute chunk generator.

    Chunks: 16 ic-blocks (w1/w3 matmuls + silu + hh) then 8 y-blocks.
    """

    def __init__(self, nc, wpool, hspool, ytpool, pg, cap, exp_silu=False):
        self.nc = nc
        self.wpool = wpool
        self.hspool = hspool
        self.ytpool = ytpool
        self.pg = pg
        self.cap = cap
        self.exp_silu = exp_silu  # silu via exp table (avoids ACT table swaps)
        self.gen = None

    def setup_loads_z(self):
        """Declare tensors + load z (cheap, needed first)."""
        nc, wpool, cap = self.nc, self.wpool, self.cap
        # w1/w3 sliced by ic-chunk, w2 by hc-chunk, so the first matmuls
        # only wait for a 128-column slice instead of the full 2MB tensor.
        self.zeT = nc.dram_tensor("zeT", [128, NCI, cap], DT.float8e4,
                                  kind="ExternalInput")
        self.w1T = nc.dram_tensor("w1T", [128, NIC, NCI, 128], DT.float8e4,
                                  kind="ExternalInput")
        self.w3T = nc.dram_tensor("w3T", [128, NIC, NCI, 128], DT.float8e4,
                                  kind="ExternalInput")
        self.w2T = nc.dram_tensor("w2T", [128, NCI, NIC, 128], DT.float8e4,
                                  kind="ExternalInput")
        self.yT = nc.dram_tensor("yT", [128, NCI, cap], DT.float8e4,
                                 kind="ExternalOutput")
        self.zcs = wpool.tile([128, NCI, cap], DT.float8e4, name="zcs", tag="zcs")
        self.w1sb = wpool.tile([128, NIC, NCI, 128], DT.float8e4,
                               name="w1sb", tag="w1sb")
        self.w3sb = wpool.tile([128, NIC, NCI, 128], DT.float8e4,
                               name="w3sb", tag="w3sb")
        self.w2sb = wpool.tile([128, NCI, NIC, 128], DT.float8e4,
                               name="w2sb", tag="w2sb")
        nc.sync.dma_start(self.zcs[:], self.zeT[:, :, :])
        self.hh8 = wpool.tile([128, NIC, cap], DT.float8e4, name="hh8", tag="hh8")

    def setup_loads_w(self):
        """Stream the weight slices (emitted after attention's reshuffle
        DMAs so they don't block the scores pipeline on the SP queue)."""
        nc = self.nc
        for ic in range(NIC):
            nc.sync.dma_start(self.w1sb[:, ic], self.w1T[:, ic])
            nc.sync.dma_start(self.w3sb[:, ic], self.w3T[:, ic])
        for hc in range(NCI):
            nc.sync.dma_start(self.w2sb[:, hc], self.w2T[:, hc])
        self.gen = self._chunks()

    def setup_loads(self):
        self.setup_loads_z()
        self.setup_loads_w()

    def drain(self, n):
        if self.gen is None:
            return
        done = object()
        for _ in range(n):
            if next(self.gen, done) is done:
                break

    def _chunks(self):
        nc, pg, cap = self.nc, self.pg, self.cap
        zcs, w1sb, w3sb, w2sb, hh8 = (self.zcs, self.w1sb, self.w3sb,
                                      self.w2sb, self.hh8)

        cch = _capacity_chunks(cap)
        # 16 ic-blocks: hp = w1.z, gp = w3.z, hh = silu(hp)*gp
        for ic in range(NIC):
            hs = self.hspool.tile([128, cap], DT.bfloat16, tag="hs", name="hs")
            vv = (self.hspool.tile([128, cap], DT.bfloat16, tag="vv", name="vv")
                  if self.exp_silu else None)
            for (o, ln) in cch:
                hp = pg.tile([128, 512], DT.float32, tag="pg", name="hp")
                for i in range(NCI // 2):
                    nc.tensor.matmul(
                        hp[:, 0:ln],
                        w1sb[:, ic, 2 * i:2 * i + 2, :],
                        zcs[:, 2 * i:2 * i + 2, o:o + ln],
                        start=(i == 0), stop=(i == NCI // 2 - 1),
                        perf_mode=PM.DoubleRow,
                    )
                gp = pg.tile([128, 512], DT.float32, tag="pg", name="gp")
                for i in range(NCI // 2):
                    nc.tensor.matmul(
                        gp[:, 0:ln],
                        w3sb[:, ic, 2 * i:2 * i + 2, :],
                        zcs[:, 2 * i:2 * i + 2, o:o + ln],
                        start=(i == 0), stop=(i == NCI // 2 - 1),
                        perf_mode=PM.DoubleRow,
                    )
                if self.exp_silu:
                    # silu(h) = h*(tanh(h/2)+1)/2 -- tanh shares the ACT
                    # table set with Exp, so no table reloads between the
                    # attention exp stream and the MoE activation.
                    # t = tanh(h/2) [ACT], tp1 = (t+1)/(2*LAM) [Pool],
                    # s = hp*tp1 = silu-ish [DVE], hh8 = s*gp [DVE]
                    nc.scalar.activation(hs[:, o:o + ln], hp[:, 0:ln], AF.Tanh,
                                         scale=0.5 / LAM)
                    ww = self.hspool.tile([128, 512], DT.bfloat16, tag="ww",
                                          name="ww")
                    nc.gpsimd.tensor_scalar(
                        ww[:, 0:ln], hs[:, o:o + ln], 1.0, 0.5 / LAM,
                        ALU.add, ALU.mult)
                    nc.vector.tensor_tensor(
                        vv[:, o:o + ln], hp[:, 0:ln], ww[:, 0:ln], ALU.mult)
                    nc.vector.tensor_tensor(
                        hh8[:, ic, o:o + ln], vv[:, o:o + ln], gp[:, 0:ln],
                        ALU.mult)
                else:
                    nc.scalar.activation(hs[:, o:o + ln], hp[:, 0:ln], AF.Silu,
                                         scale=1.0 / LAM)
                    # hh8 = LAM3 * hh_true (w3 carries LAM3, hs is true silu)
                    nc.vector.tensor_tensor(
                        hh8[:, ic, o:o + ln], gp[:, 0:ln], hs[:, o:o + ln],
                        ALU.mult)
            yield

        # 8 y-blocks: y = w2.hh, scaled to true fp8
        for hc in range(NCI):
            yt = self.ytpool.tile([128, cap], DT.float8e4, tag="yt", name="yt")
            for (o, ln) in cch:
                yp = pg.tile([128, 512], DT.float32, tag="pg", name="yp")
                for i in range(NIC // 2):
                    nc.tensor.matmul(
                        yp[:, 0:ln],
                        w2sb[:, hc, 2 * i:2 * i + 2, :],
                        hh8[:, 2 * i:2 * i + 2, o:o + ln],
                        start=(i == 0), stop=(i == NIC // 2 - 1),
                        perf_mode=PM.DoubleRow,
                    )
                nc.vector.tensor_scalar(
                    yt[:, o:o + ln], yp[:, 0:ln], 1.0 / (LAM3 * LAM), None,
                    ALU.mult)
            nc.sync.dma_start(self.yT[:, hc, :], yt[:])
            yield


def _emit_attn(nc, tc, pools, moe, moe_per_block):
    """Attention for one batch, 2 heads per core (128-wide head slice)."""
    (wpool, qkpool, vpool, ptpool, aopool, rcpool, hpool, drpool,
     pp, av, pg) = pools
    if moe_per_block is None:
        moe_per_block = [0, 0, 0, 0]

    xd = nc.dram_tensor("xd", [128, NCI, S], DT.float8e4, kind="ExternalInput")
    wqd = nc.dram_tensor("wqd", [128, NCI, 128], DT.float8e4, kind="ExternalInput")
    wkd = nc.dram_tensor("wkd", [128, NCI, 128], DT.float8e4, kind="ExternalInput")
    wvd = nc.dram_tensor("wvd", [128, NCI, 128], DT.float8e4, kind="ExternalInput")
    wod = nc.dram_tensor("wod", [64, 2, H], DT.float8e4, kind="ExternalInput")
    h1p = nc.dram_tensor("h1p", [S, H], DT.bfloat16, kind="ExternalOutput")

    # ---- input loads (xs in tq-quarters for early start) ----
    xs = wpool.tile([128, NCI, S], DT.float8e4, name="xs", tag="xs")
    nc.sync.dma_start(xs[:, :, 0:512], xd[:, :, 0:512])
    wq_sb = wpool.tile([128, NCI, 128], DT.float8e4)
    nc.sync.dma_start(wq_sb[:], wqd[:, :, :])
    wk_sb = wpool.tile([128, NCI, 128], DT.float8e4)
    nc.sync.dma_start(wk_sb[:], wkd[:, :, :])
    for tq in range(1, 4):
        nc.sync.dma_start(xs[:, :, tq * 512:(tq + 1) * 512],
                          xd[:, :, tq * 512:(tq + 1) * 512])
    wv_sb = wpool.tile([128, NCI, 128], DT.float8e4)
    nc.sync.dma_start(wv_sb[:], wvd[:, :, :])
    wo8 = wpool.tile([64, 2, H], DT.float8e4)
    nc.sync.dma_start(wo8[:], wod[:, :, :])
    if moe is not None:
        moe.setup_loads_z()

    q8sb = qkpool.tile([128, S], DT.float8e4, name="q8sb", tag="q8sb")
    k8sb = qkpool.tile([128, S], DT.float8e4, name="k8sb", tag="k8sb")
    v8 = vpool.tile([128, NTK, 2, 80], DT.float8e4, name="v8", tag="v8")
    nc.vector.memset(v8[:, :, :, 64:65], 1.0)

    # ---- q/k projections: psum [128ds, 1024] = q|k for one tq chunk.
    # q-copies on DVE, k-copies on ACT (Copy shares the exp table set) so
    # the two evac streams run in parallel and qt8/kt8 are ready sooner.
    for tq in range(4):
        pq = pp.tile([128, 1024], DT.float32, tag="pp", name="pq")
        for i in range(NCI // 2):
            nc.tensor.matmul(
                pq[:, 0:512],
                wq_sb[:, 2 * i:2 * i + 2, :],
                xs[:, 2 * i:2 * i + 2, tq * 512:(tq + 1) * 512],
                start=(i == 0), stop=(i == NCI // 2 - 1),
                perf_mode=PM.DoubleRow,
            )
            nc.tensor.matmul(
                pq[:, 512:1024],
                wk_sb[:, 2 * i:2 * i + 2, :],
                xs[:, 2 * i:2 * i + 2, tq * 512:(tq + 1) * 512],
                start=(i == 0), stop=(i == NCI // 2 - 1),
                perf_mode=PM.DoubleRow,
            )
        nc.vector.tensor_copy(q8sb[:, tq * 512:(tq + 1) * 512], pq[:, 0:512])
        nc.scalar.activation(k8sb[:, tq * 512:(tq + 1) * 512],
                             pq[:, 512:1024], AF.Copy)

    def vproj(tk):
        # v projection: psum [128t, 128ds] for one k-tile (pg pool; the av
        # pool holds live AV accumulators and must not be shared here)
        pv = pg.tile([128, 512], DT.float32, tag="pg", name="pv")
        for i in range(NCI // 2):
            nc.tensor.matmul(
                pv[:, 0:128],
                xs[:, 2 * i:2 * i + 2, tk * 128:(tk + 1) * 128],
                wv_sb[:, 2 * i:2 * i + 2, :],
                start=(i == 0), stop=(i == NCI // 2 - 1),
                perf_mode=PM.DoubleRow,
            )
        nc.vector.tensor_copy(
            v8[:, tk, :, 0:64],
            pv[:, 0:128].rearrange("p (h d) -> p h d", h=2))

    # ---- reshuffle q/k into DoubleRow-32 layout [32, head, slot, S] via a
    # DRAM round-trip (cross-partition remap is not expressible in SBUF APs)
    qt8 = qkpool.tile([32, 2, 2, S], DT.float8e4, name="qt8", tag="qt8")
    kt8 = qkpool.tile([32, 2, 2, S], DT.float8e4, name="kt8", tag="kt8")
    qstage = drpool.tile([128, S], DT.float8e4)
    kstage = drpool.tile([128, S], DT.float8e4)
    # split the round-trip in tq halves: block 0 only needs columns 0:1024.
    # k-side goes through the DVE DMA queue so q/k chains run in parallel.
    for lo, hi in ((0, 1024), (1024, 2048)):
        nc.sync.dma_start(qstage[:, lo:hi], q8sb[:, lo:hi])
        nc.scalar.dma_start(kstage[:, lo:hi], k8sb[:, lo:hi])
        nc.sync.dma_start(
            qt8[:, :, :, lo:hi],
            qstage[:, lo:hi].rearrange("(h s p) t -> p h s t", h=2, s=2))
        nc.scalar.dma_start(
            kt8[:, :, :, lo:hi],
            kstage[:, lo:hi].rearrange("(h s p) t -> p h s t", h=2, s=2))
    if moe is not None:
        moe.setup_loads_w()

    aoT = aopool.tile([64, 2, S], DT.float8e4, name="aoT", tag="aoT")

    def drain_moe(n):
        if moe is not None:
            moe.drain(n)

    def oproj(t):
        po = pg.tile([128, 512], DT.float32, tag="pg", name="po")
        po2 = pg.tile([128, 512], DT.float32, tag="pg", name="po2")
        nc.tensor.matmul(po[:], aoT[:, :, t * 128:(t + 1) * 128],
                         wo8[:, :, 0:512], start=True, stop=True,
                         perf_mode=PM.DoubleRow)
        nc.tensor.matmul(po2[:], aoT[:, :, t * 128:(t + 1) * 128],
                         wo8[:, :, 512:1024], start=True, stop=True,
                         perf_mode=PM.DoubleRow)
        ht = hpool.tile([128, H], DT.bfloat16, tag="ht", name="ht")
        nc.vector.tensor_scalar(ht[:, 0:512], po[:], OPROJ_SCALE, None, ALU.mult)
        nc.scalar.activation(ht[:, 512:1024], po2[:], AF.Copy, scale=OPROJ_SCALE)
        nc.sync.dma_start(h1p[t * 128:(t + 1) * 128, :], ht[:])

    # ---- attend blocks: (h0,tqh0), (h1,tqh0), (h0,tqh1), (h1,tqh1) ----
    pending_av7 = None  # (h, tqh, avq tiles, pt tiles) awaiting last AV pair

    def finish_block(blk):
        h, tqh, avq, pts = blk
        # last AV pair (pair 7) + normalize both 512-chunks
        for qq in range(2):
            nc.tensor.matmul(
                avq[qq][0:65, :],
                v8[:, 14:16, h, 0:65],
                pts[7][:, :, qq * 512:(qq + 1) * 512],
                start=False, stop=True, perf_mode=PM.DoubleRow,
            )
        for qq in range(2):
            rc = rcpool.tile([1, 512], DT.float32, tag="rc", name="rc")
            nc.vector.reciprocal(rc[0:1, :], avq[qq][64:65, :])
            rb = rcpool.tile([64, 512], DT.float32, tag="rb", name="rb")
            nc.gpsimd.partition_broadcast(rb[:], rc[0:1, :])
            nc.vector.tensor_tensor(
                aoT[:, h, tqh * 1024 + qq * 512:tqh * 1024 + (qq + 1) * 512],
                avq[qq][0:64, :], rb[:], ALU.mult)

    # fine-grained fill schedule: (bi, kc) -> list of work closures run
    # right after that kc's exp is emitted, keeping PE fed under the
    # ACT-bound exp stream without starving it.
    sched = defaultdict(list)
    for kc in range(8):
        sched[(0, kc)] += [lambda tk=2 * kc: vproj(tk),
                           lambda tk=2 * kc + 1: vproj(tk)]
    for bi, kc in ([(0, 8), (0, 11), (0, 14)]
                   + [(1, kc) for kc in range(1, 15, 2)]
                   + [(2, kc) for kc in range(1, 13, 2)]):
        sched[(bi, kc)].append(lambda: drain_moe(1))
    for t, kc in enumerate((2, 4, 6, 8, 10, 12, 14, 15)):
        sched[(2, kc)].append(lambda t=t: oproj(t))
    for kc in range(0, 8):
        sched[(3, kc)].append(lambda: drain_moe(1))

    for bi, (h, tqh) in enumerate(((0, 0), (1, 0), (0, 1), (1, 1))):
        avq = [av.tile([65, 512], DT.float32, tag="av", name=f"av{qq}")
               for qq in range(2)]
        pts = {}
        for kc in range(NTK):
            pi = kc // 2
            if kc % 2 == 0:
                pts[pi] = ptpool.tile([128, 2, 1024], DT.float8e4, tag="pt",
                                      name=f"pt{pi}")
            sc = pp.tile([128, 1024], DT.float32, tag="pp", name="sc")
            for i in range(2):
                nc.tensor.matmul(
                    sc[:, i * 512:(i + 1) * 512],
                    kt8[:, h, :, kc * 128:(kc + 1) * 128],
                    qt8[:, h, :, tqh * 1024 + i * 512:tqh * 1024 + (i + 1) * 512],
                    start=True, stop=True, perf_mode=PM.DoubleRow,
                )
            nc.scalar.activation(pts[pi][:, kc % 2, :], sc[:], AF.Exp,
                                 scale=EXP_SCALE)
            if kc == 1 and pending_av7 is not None:
                finish_block(pending_av7)
                pending_av7 = None
            # AV pairs 0..5, one behind the exp stream
            if kc >= 3 and kc % 2 == 1 and (kc - 3) // 2 <= 5:
                pi_av = (kc - 3) // 2
                for qq in range(2):
                    nc.tensor.matmul(
                        avq[qq][0:65, :],
                        v8[:, 2 * pi_av:2 * pi_av + 2, h, 0:65],
                        pts[pi_av][:, :, qq * 512:(qq + 1) * 512],
                        start=(pi_av == 0), stop=False,
                        perf_mode=PM.DoubleRow,
                    )
            for work in sched.get((bi, kc), ()):
                work()
        # AV pair 6 (pair6 = kc12,13 -> ready)
        for qq in range(2):
            nc.tensor.matmul(
                avq[qq][0:65, :],
                v8[:, 12:14, h, 0:65],
                pts[6][:, :, qq * 512:(qq + 1) * 512],
                start=False, stop=False, perf_mode=PM.DoubleRow,
            )
        pending_av7 = (h, tqh, avq, pts)

    finish_block(pending_av7)
    for t in range(8, 16):
        oproj(t)
    drain_moe(100)  # any remaining moe chunks


def _pools(stack, tc):
    names = [("wpool", 1, None), ("qk", 1, None), ("vp", 1, None),
             ("pt", 3, None), ("ao", 1, None), ("rc", 4, None),
             ("hout", 3, None), ("dram", 2, "DRAM"),
             ("pp", 2, "PSUM"), ("av", 2, "PSUM"), ("pg", 2, "PSUM")]
    out = []
    for name, bufs, space in names:
        kw = {"space": space} if space else {}
        out.append(stack.enter_context(tc.tile_pool(name=name, bufs=bufs, **kw)))
    return out


def _build_l1():
    from contextlib import ExitStack
    nc = bacc.Bacc("TRN2", target_bir_lowering=False, debug=False,
                   num_devices=NCORES)
    with tile.TileContext(nc) as tc, \
         nc.allow_low_precision(reason="fp8 kernel by design"), \
         ExitStack() as stack:
        pools = _pools(stack, tc)
        _emit_attn(nc, tc, pools, None, None)
    nc.compile()
    nc.finalize()
    return nc


def _build_l2(cap):
    from contextlib import ExitStack
    nc = bacc.Bacc("TRN2", target_bir_lowering=False, debug=False,
                   num_devices=NCORES)
    with tile.TileContext(nc) as tc, \
         nc.allow_low_precision(reason="fp8 kernel by design"), \
         ExitStack() as stack:
        pools = _pools(stack, tc)
        hspool = stack.enter_context(tc.tile_pool(name="hs", bufs=4))
        ytpool = stack.enter_context(tc.tile_pool(name="yt", bufs=2))
        moe = _MoeEmitter(nc, pools[0], hspool, ytpool, pools[10], cap,
                          exp_silu=True)
        _emit_attn(nc, tc, pools, moe, [6, 6, 4, 8])
    nc.compile()
    nc.finalize()
    return nc


def _build_l3(cap):
    nc = bacc.Bacc("TRN2", target_bir_lowering=False, debug=False,
                   num_devices=NCORES)
    with tile.TileContext(nc) as tc, \
         nc.allow_low_precision(reason="fp8 kernel by design"):
        with tc.tile_pool(name="wpool", bufs=1) as wpool, \
             tc.tile_pool(name="hs", bufs=2) as hspool, \
             tc.tile_pool(name="yt", bufs=4) as ytpool, \
             tc.tile_pool(name="pg", bufs=4, space="PSUM") as pg:
            moe = _MoeEmitter(nc, wpool, hspool, ytpool, pg, cap)
            moe.setup_loads()
            moe.drain(100)
    nc.compile()
    nc.finalize()
    return nc


def _get(name, builder, *args):
    if name not in _CACHE:
        _CACHE[name] = builder(*args)
    return _CACHE[name]


def _rmsnorm(x, w):
    xf = x.astype(np.float32)
    rms = 1.0 / np.sqrt((xf * xf).mean(axis=-1, keepdims=True) + EPS)
    return (xf * rms) * w.astype(np.float32)


def _f8(x):
    return np.clip(np.asarray(x, np.float32), -240.0, 240.0).astype(F8)


def _attn_inmaps(zb, wq, wk, wv, wo):
    """Per-core input maps for one batch's attention launch."""
    zT = np.ascontiguousarray(zb.T)                     # [H, S]
    xd = _f8(zT.reshape(NCI, 128, S).transpose(1, 0, 2))  # [128, NCI, S]
    maps = []
    for c in range(NCORES):
        rows = slice(c * 128, (c + 1) * 128)            # head slice outputs
        # wq_sb[p, hc, j] = LAM * wq[c*128 + j, hc*128 + p]
        wqs = _f8(LAM * wq[rows].T.reshape(NCI, 128, 128).transpose(1, 0, 2))
        wks = _f8(LAM * wk[rows].T.reshape(NCI, 128, 128).transpose(1, 0, 2))
        wvs = _f8(LAM * wv[rows].T.reshape(NCI, 128, 128).transpose(1, 0, 2))
        # wo8[d, h, :] = LAM * wo[:, c*128 + h*64 + d]  (O-proj contracts ds)
        wos = _f8(LAM * wo[:, rows].T.reshape(2, 64, H).transpose(1, 0, 2))
        maps.append({"xd": xd, "wqd": wqs, "wkd": wks, "wvd": wvs, "wod": wos})
    return maps


def _route(h1, ln2_w, gate_w):
    z = _rmsnorm(h1, ln2_w)
    logits = (z.astype(np.float64) @ gate_w.T.astype(np.float64)).astype(np.float32)
    order = np.argsort(-logits, axis=-1, kind="stable")
    sel = order[:, :2]
    vals = np.take_along_axis(logits, sel, axis=-1).astype(np.float32)
    mx = vals.max(axis=-1, keepdims=True)
    ex = np.exp(vals - mx)
    rw = (ex / ex.sum(axis=-1, keepdims=True)).astype(np.float32)
    idx_lists = []
    for e in range(E):
        m = (sel == e)
        tok = np.nonzero(m.any(axis=-1))[0]
        wgt = np.where(m, rw, 0.0).sum(axis=-1)[tok]
        idx_lists.append((tok, wgt.astype(np.float32)))
    return z, idx_lists


def _moe_inmaps(z, idx_lists, w1, w2, w3, cap):
    zT = _f8(z.T)                                       # [H, Sb]
    maps = []
    for e in range(E):
        tok, _ = idx_lists[e]
        zeT = np.zeros((H, cap), F8)
        zeT[:, :len(tok)] = zT[:, tok]
        maps.append({
            "zeT": np.ascontiguousarray(
                zeT.reshape(NCI, 128, cap).transpose(1, 0, 2)),
            "w1T": _f8(LAM * w1[e].T.reshape(NCI, 128, NIC, 128)
                       .transpose(1, 2, 0, 3)),
            "w3T": _f8(LAM3 * w3[e].T.reshape(NCI, 128, NIC, 128)
                       .transpose(1, 2, 0, 3)),
            "w2T": _f8(LAM * w2[e].T.reshape(NIC, 128, NCI, 128)
                       .transpose(1, 2, 0, 3)),
        })
    return maps


def _sum_h1p(x_b, res, cores):
    h1 = x_b.astype(np.float32).copy()
    for c in cores:
        h1 += np.asarray(res.results[c]["h1p"], np.float32)
    return h1


def _apply_moe(out_b, res, idx_lists, cap):
    for e in range(E):
        tok, wgt = idx_lists[e]
        y = np.asarray(res.results[e]["yT"], np.float32)  # [128, NCI, cap]
        y = y.transpose(1, 0, 2).reshape(H, cap)[:, :len(tok)]
        out_b[tok] += y.T * wgt[:, None]
    return out_b


def _cap_for(idx_lists):
    maxload = max(len(tok) for tok, _ in idx_lists)
    cap = 512
    while cap < maxload:
        cap += 64
    return cap


def kernel(x, ln1_w, ln2_w, wq, wk, wv, wo, gate_w, w1, w2, w3):
    global LAST_RESULTS
    LAST_RESULTS = []
    x = np.asarray(x, np.float32)
    wq, wk, wv, wo = (np.asarray(a, np.float32) for a in (wq, wk, wv, wo))
    gate_w = np.asarray(gate_w, np.float32)
    w1, w2, w3 = (np.asarray(a, np.float32) for a in (w1, w2, w3))
    ln1_w = np.asarray(ln1_w, np.float32)
    ln2_w = np.asarray(ln2_w, np.float32)

    xf = x.reshape(T, H)
    z1 = _rmsnorm(xf, ln1_w)

    # ---- L1: attention(batch 0) ----
    nc1 = _get("l1", _build_l1)
    in1 = _attn_inmaps(z1[0:S], wq, wk, wv, wo)
    res1 = run_bass_kernel_spmd(nc1, in1, core_ids=list(range(NCORES)),
                                trace=TRACE)
    LAST_RESULTS.append(res1)
    h1_b0 = _sum_h1p(xf[0:S], res1, range(NCORES))
    z2_b0, idx0 = _route(h1_b0, ln2_w, gate_w)
    cap0 = _cap_for(idx0)

    # ---- L2: attention(batch 1) + MoE(batch 0) ----
    nc2 = _get(f"l2_{cap0}", _build_l2, cap0)
    in2a = _attn_inmaps(z1[S:T], wq, wk, wv, wo)
    in2m = _moe_inmaps(z2_b0, idx0, w1, w2, w3, cap0)
    in2 = [dict(**a, **m) for a, m in zip(in2a, in2m)]
    res2 = run_bass_kernel_spmd(nc2, in2, core_ids=list(range(NCORES)),
                                trace=TRACE)
    LAST_RESULTS.append(res2)
    h1_b1 = _sum_h1p(xf[S:T], res2, range(NCORES))
    out_b0 = _apply_moe(h1_b0.copy(), res2, idx0, cap0)
    z2_b1, idx1 = _route(h1_b1, ln2_w, gate_w)
    cap1 = _cap_for(idx1)

    # ---- L3: MoE(batch 1) ----
    nc3 = _get(f"l3_{cap1}", _build_l3, cap1)
    in3 = _moe_inmaps(z2_b1, idx1, w1, w2, w3, cap1)
    res3 = run_bass_kernel_spmd(nc3, in3, core_ids=list(range(NCORES)),
                                trace=TRACE)
    LAST_RESULTS.append(res3)
    out_b1 = _apply_moe(h1_b1.copy(), res3, idx1, cap1)

    return np.concatenate([out_b0, out_b1], 0).reshape(B, S, H).astype(np.float32)


# revision 45
# speedup vs baseline: 1.0031x; 1.0031x over previous
"""Trainium2 Bass kernel for a Mixtral decoder layer (attention + top-2 MoE).

3-launch pipeline over 8 NeuronCores, all heavy matmuls in fp8e4 DoubleRow
(2 fp8 weights/PE-cell, 256-wide contraction, 0.5 cyc/row):

  L1: attention(batch 0), 8-way head-sharded (2 heads / core). Each core
      computes q/k/v projections for its 128-wide head slice, exp-softmax
      scores (ACT-bound), AV with a ones-column denominator (M=65 DoubleRow),
      and a partial O-projection -> h1p [S, H] bf16. Host sums partials.
  host: residual + rmsnorm + exact fp32/64 top-2 routing for batch 0 (free).
  L2: attention(batch 1) + expert-parallel MoE FFN(batch 0) interleaved on
      the same cores: MoE matmul slices fill the PE idle time under the
      ACT exp shadow.
  host: routing for batch 1.
  L3: MoE FFN(batch 1).

Scales: weights are pre-scaled into fp8-friendly ranges on the host
(lam=64 for wq/wk/wv/wo/w1/w2, lam=8 for w3) and descaled on-device by
folding into the exp scale / silu scale / output copies.
"""
import os
import sys
from collections import defaultdict

import numpy as np
import ml_dtypes

for _p in ("/root/.axon_site", "/root/.axon_site/_ro/trn_rl_repo", "/opt/trn_rl_repo"):
    if os.path.isdir(_p) and _p not in sys.path:
        sys.path.append(_p)

import concourse.tile as tile
from concourse import bacc, mybir
from concourse.bass_utils import run_bass_kernel_spmd

BF16 = ml_dtypes.bfloat16
F8 = ml_dtypes.float8_e4m3fn
AF = mybir.ActivationFunctionType
ALU = mybir.AluOpType
DT = mybir.dt
PM = mybir.MatmulPerfMode

H = 1024
S = 2048
B = 2
NH = 16
D = 64
E = 8
I = 2048
T = B * S
EPS = 1e-5
NCORES = 8

NCI = H // 128        # 8 contraction chunks of the hidden dim
NTK = S // 128        # 16 k-tiles
NIC = I // 128        # 16 chunks of the FFN dim

LAM = 64.0            # wq/wk/wv/wo/w1/w2 fp8 scale
LAM3 = 8.0            # w3 fp8 scale
EXP_SCALE = 0.125 / (LAM * LAM)
OPROJ_SCALE = 1.0 / (LAM * LAM)   # ao carries LAM, wo8 carries LAM
Y_SCALE = 1.0 / (LAM3 * LAM * LAM)  # hh carries LAM3*? see emit_moe

_CACHE = {}
LAST_RESULTS = []
TRACE = os.environ.get("KERNEL_TRACE", "0") == "1"


def _capacity_chunks(cap):
    out, o = [], 0
    while o < cap:
        ln = min(512, cap - o)
        out.append((o, ln))
        o += ln
    return out


class _MoeEmitter:
    """MoE FFN for one expert-core: load setup + compute chunk generator.

    Chunks: 16 ic-blocks (w1/w3 matmuls + silu + hh) then 8 y-blocks.
    """

    def __init__(self, nc, wpool, hspool, ytpool, pg, cap, exp_silu=False):
        self.nc = nc
        self.wpool = wpool
        self.hspool = hspool
        self.ytpool = ytpool
        self.pg = pg
        self.cap = cap
        self.exp_silu = exp_silu  # silu via exp table (avoids ACT table swaps)
        self.gen = None

    def setup_loads_z(self):
        """Declare tensors + load z (cheap, needed first)."""
        nc, wpool, cap = self.nc, self.wpool, self.cap
        # w1/w3 sliced by ic-chunk, w2 by hc-chunk, so the first matmuls
        # only wait for a 128-column slice instead of the full 2MB tensor.
        self.zeT = nc.dram_tensor("zeT", [128, NCI, cap], DT.float8e4,
                                  kind="ExternalInput")
        self.w1T = nc.dram_tensor("w1T", [128, NIC, NCI, 128], DT.float8e4,
                                  kind="ExternalInput")
        self.w3T = nc.dram_tensor("w3T", [128, NIC, NCI, 128], DT.float8e4,
                                  kind="ExternalInput")
        self.w2T = nc.dram_tensor("w2T", [128, NCI, NIC, 128], DT.float8e4,
                                  kind="ExternalInput")
        self.yT = nc.dram_tensor("yT", [128, NCI, cap], DT.float8e4,
                                 kind="ExternalOutput")
        self.zcs = wpool.tile([128, NCI, cap], DT.float8e4, name="zcs", tag="zcs")
        self.w1sb = wpool.tile([128, NIC, NCI, 128], DT.float8e4,
                               name="w1sb", tag="w1sb")
        self.w3sb = wpool.tile([128, NIC, NCI, 128], DT.float8e4,
                               name="w3sb", tag="w3sb")
        self.w2sb = wpool.tile([128, NCI, NIC, 128], DT.float8e4,
                               name="w2sb", tag="w2sb")
        nc.sync.dma_start(self.zcs[:], self.zeT[:, :, :])
        self.hh8 = wpool.tile([128, NIC, cap], DT.float8e4, name="hh8", tag="hh8")

    def setup_loads_w(self):
        """Stream the weight slices (emitted after attention's reshuffle
        DMAs so they don't block the scores pipeline on the SP queue)."""
        nc = self.nc
        for ic in range(NIC):
            nc.sync.dma_start(self.w1sb[:, ic], self.w1T[:, ic])
            nc.sync.dma_start(self.w3sb[:, ic], self.w3T[:, ic])
        for hc in range(NCI):
            nc.sync.dma_start(self.w2sb[:, hc], self.w2T[:, hc])
        self.gen = self._chunks()

    def setup_loads(self):
        self.setup_loads_z()
        self.setup_loads_w()

    def drain(self, n):
        if self.gen is None:
            return
        done = object()
        for _ in range(n):
            if next(self.gen, done) is done:
                break

    def _chunks(self):
        nc, pg, cap = self.nc, self.pg, self.cap
        zcs, w1sb, w3sb, w2sb, hh8 = (self.zcs, self.w1sb, self.w3sb,
                                      self.w2sb, self.hh8)

        cch = _capacity_chunks(cap)
        # 16 ic-blocks: hp = w1.z, gp = w3.z, hh = silu(hp)*gp.
        # Yield after every cap-chunk: insertion slices must stay well under
        # PE's 32-instruction reorder window or they stall the exp stream.
        for ic in range(NIC):
            hs = self.hspool.tile([128, cap], DT.bfloat16, tag="hs", name="hs")
            vv = (self.hspool.tile([128, cap], DT.bfloat16, tag="vv", name="vv")
                  if self.exp_silu else None)
            for (o, ln) in cch:
                hp = pg.tile([128, 512], DT.float32, tag="pg", name="hp")
                for i in range(NCI // 2):
                    nc.tensor.matmul(
                        hp[:, 0:ln],
                        w1sb[:, ic, 2 * i:2 * i + 2, :],
                        zcs[:, 2 * i:2 * i + 2, o:o + ln],
                        start=(i == 0), stop=(i == NCI // 2 - 1),
                        perf_mode=PM.DoubleRow,
                    )
                gp = pg.tile([128, 512], DT.float32, tag="pg", name="gp")
                for i in range(NCI // 2):
                    nc.tensor.matmul(
                        gp[:, 0:ln],
                        w3sb[:, ic, 2 * i:2 * i + 2, :],
                        zcs[:, 2 * i:2 * i + 2, o:o + ln],
                        start=(i == 0), stop=(i == NCI // 2 - 1),
                        perf_mode=PM.DoubleRow,
                    )
                if self.exp_silu:
                    # silu(h) = h*(tanh(h/2)+1)/2 -- tanh shares the ACT
                    # table set with Exp, so no table reloads between the
                    # attention exp stream and the MoE activation.
                    # t = tanh(h/2) [ACT], tp1 = (t+1)/(2*LAM) [Pool],
                    # s = hp*tp1 = silu-ish [DVE], hh8 = s*gp [DVE]
                    nc.scalar.activation(hs[:, o:o + ln], hp[:, 0:ln], AF.Tanh,
                                         scale=0.5 / LAM)
                    ww = self.hspool.tile([128, 512], DT.bfloat16, tag="ww",
                                          name="ww")
                    nc.gpsimd.tensor_scalar(
                        ww[:, 0:ln], hs[:, o:o + ln], 1.0, 0.5 / LAM,
                        ALU.add, ALU.mult)
                    nc.vector.tensor_tensor(
                        vv[:, o:o + ln], hp[:, 0:ln], ww[:, 0:ln], ALU.mult)
                    nc.vector.tensor_tensor(
                        hh8[:, ic, o:o + ln], vv[:, o:o + ln], gp[:, 0:ln],
                        ALU.mult)
                else:
                    nc.scalar.activation(hs[:, o:o + ln], hp[:, 0:ln], AF.Silu,
                                         scale=1.0 / LAM)
                    # hh8 = LAM3 * hh_true (w3 carries LAM3, hs is true silu)
                    nc.vector.tensor_tensor(
                        hh8[:, ic, o:o + ln], gp[:, 0:ln], hs[:, o:o + ln],
                        ALU.mult)
                yield

        # 8 y-blocks: y = w2.hh, scaled to true fp8
        for hc in range(NCI):
            yt = self.ytpool.tile([128, cap], DT.float8e4, tag="yt", name="yt")
            for (o, ln) in cch:
                yp = pg.tile([128, 512], DT.float32, tag="pg", name="yp")
                for i in range(NIC // 2):
                    nc.tensor.matmul(
                        yp[:, 0:ln],
                        w2sb[:, hc, 2 * i:2 * i + 2, :],
                        hh8[:, 2 * i:2 * i + 2, o:o + ln],
                        start=(i == 0), stop=(i == NIC // 2 - 1),
                        perf_mode=PM.DoubleRow,
                    )
                nc.vector.tensor_scalar(
                    yt[:, o:o + ln], yp[:, 0:ln], 1.0 / (LAM3 * LAM), None,
                    ALU.mult)
                yield
            nc.sync.dma_start(self.yT[:, hc, :], yt[:])


def _emit_attn(nc, tc, pools, moe, moe_per_block):
    """Attention for one batch, 2 heads per core (128-wide head slice)."""
    (wpool, qkpool, vpool, ptpool, aopool, rcpool, hpool, drpool,
     pp, av, pg) = pools
    if moe_per_block is None:
        moe_per_block = [0, 0, 0, 0]

    xd = nc.dram_tensor("xd", [128, NCI, S], DT.float8e4, kind="ExternalInput")
    wqd = nc.dram_tensor("wqd", [128, NCI, 128], DT.float8e4, kind="ExternalInput")
    wkd = nc.dram_tensor("wkd", [128, NCI, 128], DT.float8e4, kind="ExternalInput")
    wvd = nc.dram_tensor("wvd", [128, NCI, 128], DT.float8e4, kind="ExternalInput")
    wod = nc.dram_tensor("wod", [64, 2, H], DT.float8e4, kind="ExternalInput")
    h1p = nc.dram_tensor("h1p", [S, H], DT.bfloat16, kind="ExternalOutput")

    # ---- input loads (xs in tq-quarters for early start) ----
    xs = wpool.tile([128, NCI, S], DT.float8e4, name="xs", tag="xs")
    nc.sync.dma_start(xs[:, :, 0:512], xd[:, :, 0:512])
    wq_sb = wpool.tile([128, NCI, 128], DT.float8e4)
    nc.sync.dma_start(wq_sb[:], wqd[:, :, :])
    wk_sb = wpool.tile([128, NCI, 128], DT.float8e4)
    nc.sync.dma_start(wk_sb[:], wkd[:, :, :])
    for tq in range(1, 4):
        nc.sync.dma_start(xs[:, :, tq * 512:(tq + 1) * 512],
                          xd[:, :, tq * 512:(tq + 1) * 512])
    wv_sb = wpool.tile([128, NCI, 128], DT.float8e4)
    nc.sync.dma_start(wv_sb[:], wvd[:, :, :])
    wo8 = wpool.tile([64, 2, H], DT.float8e4)
    nc.sync.dma_start(wo8[:], wod[:, :, :])
    if moe is not None:
        moe.setup_loads_z()

    q8sb = qkpool.tile([128, S], DT.float8e4, name="q8sb", tag="q8sb")
    k8sb = qkpool.tile([128, S], DT.float8e4, name="k8sb", tag="k8sb")
    v8 = vpool.tile([128, NTK, 2, 80], DT.float8e4, name="v8", tag="v8")
    nc.vector.memset(v8[:, :, :, 64:65], 1.0)

    # ---- q/k projections: psum [128ds, 1024] = q|k for one tq chunk.
    # q-copies on DVE, k-copies on ACT (Copy shares the exp table set) so
    # the two evac streams run in parallel and qt8/kt8 are ready sooner.
    for tq in range(4):
        pq = pp.tile([128, 1024], DT.float32, tag="pp", name="pq")
        for i in range(NCI // 2):
            nc.tensor.matmul(
                pq[:, 0:512],
                wq_sb[:, 2 * i:2 * i + 2, :],
                xs[:, 2 * i:2 * i + 2, tq * 512:(tq + 1) * 512],
                start=(i == 0), stop=(i == NCI // 2 - 1),
                perf_mode=PM.DoubleRow,
            )
            nc.tensor.matmul(
                pq[:, 512:1024],
                wk_sb[:, 2 * i:2 * i + 2, :],
                xs[:, 2 * i:2 * i + 2, tq * 512:(tq + 1) * 512],
                start=(i == 0), stop=(i == NCI // 2 - 1),
                perf_mode=PM.DoubleRow,
            )
        nc.vector.tensor_copy(q8sb[:, tq * 512:(tq + 1) * 512], pq[:, 0:512])
        nc.scalar.activation(k8sb[:, tq * 512:(tq + 1) * 512],
                             pq[:, 512:1024], AF.Copy)

    def vproj(tk):
        # v projection: psum [128t, 128ds] for one k-tile (pg pool; the av
        # pool holds live AV accumulators and must not be shared here)
        pv = pg.tile([128, 512], DT.float32, tag="pg", name="pv")
        for i in range(NCI // 2):
            nc.tensor.matmul(
                pv[:, 0:128],
                xs[:, 2 * i:2 * i + 2, tk * 128:(tk + 1) * 128],
                wv_sb[:, 2 * i:2 * i + 2, :],
                start=(i == 0), stop=(i == NCI // 2 - 1),
                perf_mode=PM.DoubleRow,
            )
        nc.vector.tensor_copy(
            v8[:, tk, :, 0:64],
            pv[:, 0:128].rearrange("p (h d) -> p h d", h=2))

    # ---- reshuffle q/k into DoubleRow-32 layout [32, head, slot, S] via a
    # DRAM round-trip (cross-partition remap is not expressible in SBUF APs)
    qt8 = qkpool.tile([32, 2, 2, S], DT.float8e4, name="qt8", tag="qt8")
    kt8 = qkpool.tile([32, 2, 2, S], DT.float8e4, name="kt8", tag="kt8")
    qstage = drpool.tile([128, S], DT.float8e4)
    kstage = drpool.tile([128, S], DT.float8e4)
    # split the round-trip in tq halves: block 0 only needs columns 0:1024.
    # k-side goes through the DVE DMA queue so q/k chains run in parallel.
    for lo, hi in ((0, 1024), (1024, 2048)):
        nc.sync.dma_start(qstage[:, lo:hi], q8sb[:, lo:hi])
        nc.scalar.dma_start(kstage[:, lo:hi], k8sb[:, lo:hi])
        nc.sync.dma_start(
            qt8[:, :, :, lo:hi],
            qstage[:, lo:hi].rearrange("(h s p) t -> p h s t", h=2, s=2))
        nc.scalar.dma_start(
            kt8[:, :, :, lo:hi],
            kstage[:, lo:hi].rearrange("(h s p) t -> p h s t", h=2, s=2))
    if moe is not None:
        moe.setup_loads_w()

    aoT = aopool.tile([64, 2, S], DT.float8e4, name="aoT", tag="aoT")

    def drain_moe(n):
        if moe is not None:
            moe.drain(n)

    def oproj(t, tail=False):
        po = pg.tile([128, 512], DT.float32, tag="pg", name="po")
        po2 = pg.tile([128, 512], DT.float32, tag="pg", name="po2")
        nc.tensor.matmul(po[:], aoT[:, :, t * 128:(t + 1) * 128],
                         wo8[:, :, 0:512], start=True, stop=True,
                         perf_mode=PM.DoubleRow)
        nc.tensor.matmul(po2[:], aoT[:, :, t * 128:(t + 1) * 128],
                         wo8[:, :, 512:1024], start=True, stop=True,
                         perf_mode=PM.DoubleRow)
        ht = hpool.tile([128, H], DT.bfloat16, tag="ht", name="ht")
        nc.vector.tensor_scalar(ht[:, 0:512], po[:], OPROJ_SCALE, None, ALU.mult)
        if tail:
            # ACT is idle after the last exp; split the evacuations
            nc.scalar.activation(ht[:, 512:1024], po2[:], AF.Copy,
                                 scale=OPROJ_SCALE)
        else:
            nc.vector.tensor_scalar(ht[:, 512:1024], po2[:], OPROJ_SCALE, None,
                                    ALU.mult)
        nc.sync.dma_start(h1p[t * 128:(t + 1) * 128, :], ht[:])

    # ---- attend blocks: (h0,tqh0), (h1,tqh0), (h0,tqh1), (h1,tqh1) ----
    pending_av7 = None  # (h, tqh, avq tiles, pt tiles) awaiting last AV pair

    def finish_block(blk):
        h, tqh, avq, pts = blk
        # last AV pair (pair 7) + normalize both 512-chunks
        for qq in range(2):
            nc.tensor.matmul(
                avq[qq][0:65, :],
                v8[:, 14:16, h, 0:65],
                pts[7][:, :, qq * 512:(qq + 1) * 512],
                start=False, stop=True, perf_mode=PM.DoubleRow,
            )
        for qq in range(2):
            rc = rcpool.tile([1, 512], DT.float32, tag="rc", name="rc")
            nc.vector.reciprocal(rc[0:1, :], avq[qq][64:65, :])
            rb = rcpool.tile([64, 512], DT.float32, tag="rb", name="rb")
            nc.gpsimd.partition_broadcast(rb[:], rc[0:1, :])
            nc.vector.tensor_tensor(
                aoT[:, h, tqh * 1024 + qq * 512:tqh * 1024 + (qq + 1) * 512],
                avq[qq][0:64, :], rb[:], ALU.mult)

    # fine-grained fill schedule: (bi, kc) -> list of work closures run
    # right after that kc's exp is emitted, keeping PE fed under the
    # ACT-bound exp stream without starving it.
    sched = defaultdict(list)
    for kc in range(8):
        sched[(0, kc)] += [lambda tk=2 * kc: vproj(tk),
                           lambda tk=2 * kc + 1: vproj(tk)]
    if moe is not None:
        ncch = len(_capacity_chunks(moe.cap))
        ic_slots = ([(0, kc) for kc in range(8, 16)]
                    + [(1, kc) for kc in range(1, 16)]
                    + [(2, kc) for kc in range(2, 16, 2)])
        for j in range(NIC * ncch):
            bi, kc = ic_slots[j % len(ic_slots)]
            sched[(bi, kc)].append(lambda: drain_moe(1))
        for j in range(NCI * ncch):
            sched[(3, j % 8)].append(lambda: drain_moe(1))
    for t, kc in enumerate((1, 3, 5, 7, 9, 11, 13, 15)):
        sched[(2, kc)].append(lambda t=t: oproj(t))

    for bi, (h, tqh) in enumerate(((0, 0), (1, 0), (0, 1), (1, 1))):
        avq = [av.tile([65, 512], DT.float32, tag="av", name=f"av{qq}")
               for qq in range(2)]
        pts = {}
        for kc in range(NTK):
            pi = kc // 2
            if kc % 2 == 0:
                pts[pi] = ptpool.tile([128, 2, 1024], DT.float8e4, tag="pt",
                                      name=f"pt{pi}")
            sc = pp.tile([128, 1024], DT.float32, tag="pp", name="sc")
            for i in range(2):
                nc.tensor.matmul(
                    sc[:, i * 512:(i + 1) * 512],
                    kt8[:, h, :, kc * 128:(kc + 1) * 128],
                    qt8[:, h, :, tqh * 1024 + i * 512:tqh * 1024 + (i + 1) * 512],
                    start=True, stop=True, perf_mode=PM.DoubleRow,
                )
            nc.scalar.activation(pts[pi][:, kc % 2, :], sc[:], AF.Exp,
                                 scale=EXP_SCALE)
            if kc == 1 and pending_av7 is not None:
                finish_block(pending_av7)
                pending_av7 = None
            # AV pairs 0..5, one behind the exp stream
            if kc >= 3 and kc % 2 == 1 and (kc - 3) // 2 <= 5:
                pi_av = (kc - 3) // 2
                for qq in range(2):
                    nc.tensor.matmul(
                        avq[qq][0:65, :],
                        v8[:, 2 * pi_av:2 * pi_av + 2, h, 0:65],
                        pts[pi_av][:, :, qq * 512:(qq + 1) * 512],
                        start=(pi_av == 0), stop=False,
                        perf_mode=PM.DoubleRow,
                    )
            for work in sched.get((bi, kc), ()):
                work()
        # AV pair 6 (pair6 = kc12,13 -> ready)
        for qq in range(2):
            nc.tensor.matmul(
                avq[qq][0:65, :],
                v8[:, 12:14, h, 0:65],
                pts[6][:, :, qq * 512:(qq + 1) * 512],
                start=False, stop=False, perf_mode=PM.DoubleRow,
            )
        pending_av7 = (h, tqh, avq, pts)

    finish_block(pending_av7)
    for t in range(8, 16):
        oproj(t, tail=True)
    drain_moe(100)  # any remaining moe chunks


def _pools(stack, tc):
    names = [("wpool", 1, None), ("qk", 1, None), ("vp", 1, None),
             ("pt", 3, None), ("ao", 1, None), ("rc", 4, None),
             ("hout", 3, None), ("dram", 2, "DRAM"),
             ("pp", 2, "PSUM"), ("av", 2, "PSUM"), ("pg", 2, "PSUM")]
    out = []
    for name, bufs, space in names:
        kw = {"space": space} if space else {}
        out.append(stack.enter_context(tc.tile_pool(name=name, bufs=bufs, **kw)))
    return out


def _build_l1():
    from contextlib import ExitStack
    nc = bacc.Bacc("TRN2", target_bir_lowering=False, debug=False,
                   num_devices=NCORES)
    with tile.TileContext(nc) as tc, \
         nc.allow_low_precision(reason="fp8 kernel by design"), \
         ExitStack() as stack:
        pools = _pools(stack, tc)
        _emit_attn(nc, tc, pools, None, None)
    nc.compile()
    nc.finalize()
    return nc


def _build_l2(cap):
    from contextlib import ExitStack
    nc = bacc.Bacc("TRN2", target_bir_lowering=False, debug=False,
                   num_devices=NCORES)
    with tile.TileContext(nc) as tc, \
         nc.allow_low_precision(reason="fp8 kernel by design"), \
         ExitStack() as stack:
        pools = _pools(stack, tc)
        hspool = stack.enter_context(tc.tile_pool(name="hs", bufs=4))
        ytpool = stack.enter_context(tc.tile_pool(name="yt", bufs=2))
        moe = _MoeEmitter(nc, pools[0], hspool, ytpool, pools[10], cap,
                          exp_silu=True)
        _emit_attn(nc, tc, pools, moe, [6, 6, 4, 8])
    nc.compile()
    nc.finalize()
    return nc


def _build_l3(cap):
    nc = bacc.Bacc("TRN2", target_bir_lowering=False, debug=False,
                   num_devices=NCORES)
    with tile.TileContext(nc) as tc, \
         nc.allow_low_precision(reason="fp8 kernel by design"):
        with tc.tile_pool(name="wpool", bufs=1) as wpool, \
             tc.tile_pool(name="hs", bufs=2) as hspool, \
             tc.tile_pool(name="yt", bufs=4) as ytpool, \
             tc.tile_pool(name="pg", bufs=4, space="PSUM") as pg:
            moe = _MoeEmitter(nc, wpool, hspool, ytpool, pg, cap)
            moe.setup_loads()
            moe.drain(100)
    nc.compile()
    nc.finalize()
    return nc


def _get(name, builder, *args):
    if name not in _CACHE:
        _CACHE[name] = builder(*args)
    return _CACHE[name]


def _rmsnorm(x, w):
    xf = x.astype(np.float32)
    rms = 1.0 / np.sqrt((xf * xf).mean(axis=-1, keepdims=True) + EPS)
    return (xf * rms) * w.astype(np.float32)


def _f8(x):
    return np.clip(np.asarray(x, np.float32), -240.0, 240.0).astype(F8)


def _attn_inmaps(zb, wq, wk, wv, wo):
    """Per-core input maps for one batch's attention launch."""
    zT = np.ascontiguousarray(zb.T)                     # [H, S]
    xd = _f8(zT.reshape(NCI, 128, S).transpose(1, 0, 2))  # [128, NCI, S]
    maps = []
    for c in range(NCORES):
        rows = slice(c * 128, (c + 1) * 128)            # head slice outputs
        # wq_sb[p, hc, j] = LAM * wq[c*128 + j, hc*128 + p]
        wqs = _f8(LAM * wq[rows].T.reshape(NCI, 128, 128).transpose(1, 0, 2))
        wks = _f8(LAM * wk[rows].T.reshape(NCI, 128, 128).transpose(1, 0, 2))
        wvs = _f8(LAM * wv[rows].T.reshape(NCI, 128, 128).transpose(1, 0, 2))
        # wo8[d, h, :] = LAM * wo[:, c*128 + h*64 + d]  (O-proj contracts ds)
        wos = _f8(LAM * wo[:, rows].T.reshape(2, 64, H).transpose(1, 0, 2))
        maps.append({"xd": xd, "wqd": wqs, "wkd": wks, "wvd": wvs, "wod": wos})
    return maps


def _route(h1, ln2_w, gate_w):
    z = _rmsnorm(h1, ln2_w)
    logits = (z.astype(np.float64) @ gate_w.T.astype(np.float64)).astype(np.float32)
    order = np.argsort(-logits, axis=-1, kind="stable")
    sel = order[:, :2]
    vals = np.take_along_axis(logits, sel, axis=-1).astype(np.float32)
    mx = vals.max(axis=-1, keepdims=True)
    ex = np.exp(vals - mx)
    rw = (ex / ex.sum(axis=-1, keepdims=True)).astype(np.float32)
    idx_lists = []
    for e in range(E):
        m = (sel == e)
        tok = np.nonzero(m.any(axis=-1))[0]
        wgt = np.where(m, rw, 0.0).sum(axis=-1)[tok]
        idx_lists.append((tok, wgt.astype(np.float32)))
    return z, idx_lists


def _moe_inmaps(z, idx_lists, w1, w2, w3, cap):
    zT = _f8(z.T)                                       # [H, Sb]
    maps = []
    for e in range(E):
        tok, _ = idx_lists[e]
        zeT = np.zeros((H, cap), F8)
        zeT[:, :len(tok)] = zT[:, tok]
        maps.append({
            "zeT": np.ascontiguousarray(
                zeT.reshape(NCI, 128, cap).transpose(1, 0, 2)),
            "w1T": _f8(LAM * w1[e].T.reshape(NCI, 128, NIC, 128)
                       .transpose(1, 2, 0, 3)),
            "w3T": _f8(LAM3 * w3[e].T.reshape(NCI, 128, NIC, 128)
                       .transpose(1, 2, 0, 3)),
            "w2T": _f8(LAM * w2[e].T.reshape(NIC, 128, NCI, 128)
                       .transpose(1, 2, 0, 3)),
        })
    return maps


def _sum_h1p(x_b, res, cores):
    h1 = x_b.astype(np.float32).copy()
    for c in cores:
        h1 += np.asarray(res.results[c]["h1p"], np.float32)
    return h1


def _apply_moe(out_b, res, idx_lists, cap):
    for e in range(E):
        tok, wgt = idx_lists[e]
        y = np.asarray(res.results[e]["yT"], np.float32)  # [128, NCI, cap]
        y = y.transpose(1, 0, 2).reshape(H, cap)[:, :len(tok)]
        out_b[tok] += y.T * wgt[:, None]
    return out_b


def _cap_for(idx_lists):
    maxload = max(len(tok) for tok, _ in idx_lists)
    cap = 512
    while cap < maxload:
        cap += 64
    return cap


def kernel(x, ln1_w, ln2_w, wq, wk, wv, wo, gate_w, w1, w2, w3):
    global LAST_RESULTS
    LAST_RESULTS = []
    x = np.asarray(x, np.float32)
    wq, wk, wv, wo = (np.asarray(a, np.float32) for a in (wq, wk, wv, wo))
    gate_w = np.asarray(gate_w, np.float32)
    w1, w2, w3 = (np.asarray(a, np.float32) for a in (w1, w2, w3))
    ln1_w = np.asarray(ln1_w, np.float32)
    ln2_w = np.asarray(ln2_w, np.float32)

    xf = x.reshape(T, H)
    z1 = _rmsnorm(xf, ln1_w)

    # ---- L1: attention(batch 0) ----
    nc1 = _get("l1", _build_l1)
    in1 = _attn_inmaps(z1[0:S], wq, wk, wv, wo)
    res1 = run_bass_kernel_spmd(nc1, in1, core_ids=list(range(NCORES)),
                                trace=TRACE)
    LAST_RESULTS.append(res1)
    h1_b0 = _sum_h1p(xf[0:S], res1, range(NCORES))
    z2_b0, idx0 = _route(h1_b0, ln2_w, gate_w)
    cap0 = _cap_for(idx0)

    # ---- L2: attention(batch 1) + MoE(batch 0) ----
    nc2 = _get(f"l2_{cap0}", _build_l2, cap0)
    in2a = _attn_inmaps(z1[S:T], wq, wk, wv, wo)
    in2m = _moe_inmaps(z2_b0, idx0, w1, w2, w3, cap0)
    in2 = [dict(**a, **m) for a, m in zip(in2a, in2m)]
    res2 = run_bass_kernel_spmd(nc2, in2, core_ids=list(range(NCORES)),
                                trace=TRACE)
    LAST_RESULTS.append(res2)
    h1_b1 = _sum_h1p(xf[S:T], res2, range(NCORES))
    out_b0 = _apply_moe(h1_b0.copy(), res2, idx0, cap0)
    z2_b1, idx1 = _route(h1_b1, ln2_w, gate_w)
    cap1 = _cap_for(idx1)

    # ---- L3: MoE(batch 1) ----
    nc3 = _get(f"l3_{cap1}", _build_l3, cap1)
    in3 = _moe_inmaps(z2_b1, idx1, w1, w2, w3, cap1)
    res3 = run_bass_kernel_spmd(nc3, in3, core_ids=list(range(NCORES)),
                                trace=TRACE)
    LAST_RESULTS.append(res3)
    out_b1 = _apply_moe(h1_b1.copy(), res3, idx1, cap1)

    return np.concatenate([out_b0, out_b1], 0).reshape(B, S, H).astype(np.float32)
